# revision 1
# baseline (speedup 1.0000x reference)
import numpy as np
import jax
import jax.numpy as jnp
from jax.sharding import Mesh, PartitionSpec as P
from jax.experimental.shard_map import shard_map
from functools import partial

# Problem constants (nn_GCNContext): block-diagonal batch of B graphs,
# T nodes each, E_PER edges each. Edges never cross graph boundaries.
B, T, E_PER = 2048, 50, 600
IN, POS, H, OUT = 512, 64, 512, 512
N = B * T
E = B * E_PER
BN_EPS = 1e-5
NC = 8  # NeuronCores; shard whole graphs across cores (graph-level data parallel)

_compiled = None


def _build_forward(mesh):
    def fwd(xin, A, W1, b1, g1, be1, W2, b2, g2, be2, W3, b3, g3, be3, Wl, bl):
        # xin: [B/NC, T, IN+POS] local shard, A: [B/NC, T, T] local shard
        nb = xin.shape[0]

        def bn_relu(c, g, be):
            # global (cross-core) BatchNorm over all N nodes, biased variance
            m = jax.lax.psum(c.sum((0, 1)), 'i') / N
            v = jax.lax.psum(((c - m) ** 2).sum((0, 1)), 'i') / N
            return jax.nn.relu(g * (c - m) * jax.lax.rsqrt(v + BN_EPS) + be)

        def conv(h, W, b):
            hw = (h.reshape(nb * T, -1) @ W).reshape(nb, T, H)
            return jnp.einsum('gts,gsd->gtd', A, hw) + b

        x1 = bn_relu(conv(xin, W1, b1), g1, be1)
        x2 = bn_relu(conv(x1, W2, b2), g2, be2)
        x3 = bn_relu(conv(x2, W3, b3), g3, be3)
        h = x1 + x2 + x3
        out = jnp.tanh((h.reshape(nb * T, H) @ Wl) + bl)
        return out.reshape(nb, T, OUT)

    shard = P('i', None, None)
    rep = P()
    f = shard_map(
        fwd, mesh=mesh,
        in_specs=(shard, shard) + (rep,) * 14,
        out_specs=shard,
    )
    return jax.jit(f)


def kernel(**inputs):
    x = np.asarray(inputs['x'], np.float32)
    ei = np.asarray(inputs['edge_index'])
    ew = np.asarray(inputs['edge_weight'], np.float32)
    pos = np.asarray(inputs['pos'])
    posemb = np.asarray(inputs['posemb'], np.float32)

    src = ei[0].astype(np.int64)
    dst = ei[1].astype(np.int64)

    # Host-side sharding prep: symmetric-normalized degree (incl. self loops
    # of weight 1), then per-graph dense [T,T] adjacency blocks.
    deg = np.zeros(N, np.float32)
    np.add.at(deg, dst, ew)
    deg += 1.0
    dinv = (1.0 / np.sqrt(deg)).astype(np.float32)

    A = np.zeros((B, T, T), np.float32)
    np.add.at(A, (src // T, dst % T, src % T), ew * dinv[src] * dinv[dst])
    ar = np.arange(N)
    A[ar // T, ar % T, ar % T] += dinv * dinv

    xin = np.concatenate([x, posemb[pos]], axis=1).reshape(B, T, IN + POS)

    global _compiled
    devs = jax.devices()[:NC]
    mesh = Mesh(np.array(devs), ('i',))
    if _compiled is None:
        _compiled = _build_forward(mesh)

    args = [xin, A] + [np.asarray(inputs[k], np.float32) for k in
                       ('W1', 'b1', 'g1', 'be1', 'W2', 'b2', 'g2', 'be2',
                        'W3', 'b3', 'g3', 'be3', 'Wl', 'bl')]
    with mesh:
        out = _compiled(*args)
    return np.asarray(jax.device_get(out), np.float32)

